# revision 3
# baseline (speedup 1.0000x reference)
"""Bilateral filter (7x7, sigma_color=0.1) Trainium2 Bass kernel, v6.

Key structure (per core; image sharded 4H x 2W over 8 cores):
  - STX[b, (jy,r), c, xe] fp16: x-extended strip stack, ONE copy (no 7x
    o-expansion). cc center tile loaded from STX via jy=3-replicated DMA.
  - D-path with offset symmetry E(p,d) = E(p+d,-d): only o in {0,1,2,3}
    computed directly; o in {4,5,6} derived by partition-permute matmuls
    (PMm/PM0/PMp for cross-block rows) + x-shifted column slice.
  - E = prod_c Derivative_Erf(sqrt(50)*diff_c): one ACT op replaces
    Square + channel adds + Exp. The (2/sqrt(pi))^3 factor cancels in
    num/den.
  - Spatial weights g folded into per-o collapse matmul weights (COL),
    so exp needs no bias and derived E tiles are directly reusable.
  - Boundary terms whose symmetric source lies outside the core's rows
    are computed directly on a 36-partition fringe tile and accumulated
    into the first/last block's psum.
  - psum [18, 2560] = [V0 V1 V2 den], fp16 evac, gather 7 blocks, then
    reciprocal_approx_fast + gpsimd mults.
"""

import math

import numpy as np

import concourse.bass as bass
import concourse.bacc as bacc
import concourse.mybir as mybir
from concourse.tile import TileContext
from concourse.bass import AP

F16 = np.float16
F32 = np.float32

H, W, C = 720, 1280, 3
K = 7
PAD = 3
SIGMA_COLOR = 0.1
EXP_SCALE = -1.0 / (2.0 * SIGMA_COLOR**2)  # -50.0
ERF_SCALE = math.sqrt(-EXP_SCALE)  # sqrt(50)
NORM_COLOR = 1.0 / (2.0 * math.pi * SIGMA_COLOR**2)

HSH, WSH = 4, 2
RB = 18
JY = 7
P = JY * RB  # 126
XW = W // WSH  # 640
EXT = 644    # E-tile group width (cols [0, 643) valid, 643 junk pad)
SW = 652     # stx width
N_CORES = 8
MMN = 512

# fringe combos: (r, dy) pairs whose symmetric source row is outside the core
_TOP_RD = [(0, -1), (0, -2), (0, -3), (1, -2), (1, -3), (2, -3)]
_BOT_RD = [(17, 1), (17, 2), (17, 3), (16, 2), (16, 3), (15, 3)]


def _alu(name):
    return getattr(mybir.AluOpType, name)


ACT_FUNC = "Derivative_Erf"  # smoke test overrides (must be an even fn)
ACT_SCALE = ERF_SCALE


def build_nc(nb: int, xw: int = XW):
    dt = mybir.dt
    nc = bacc.Bacc("TRN2", debug=False)
    ext = EXT
    sw = xw + 12
    cg = C * ext      # 1932
    fdsub = 4 * cg    # 7728
    vgw = C * xw      # 1920 packed V per o
    fw = vgw + xw     # 2560 psum width

    STX = nc.dram_tensor("STX", [nb, P, C, sw], dt.float16, kind="ExternalInput")
    COL = nc.dram_tensor("COL", [P, K, RB], dt.float16, kind="ExternalInput")
    PM = nc.dram_tensor("PM", [P, 3, P], dt.float16, kind="ExternalInput")
    FRS = nc.dram_tensor("FRS", [36, C, ext], dt.float16, kind="ExternalInput")
    FRC = nc.dram_tensor("FRC", [36, C, ext], dt.float16, kind="ExternalInput")
    FCOL = nc.dram_tensor("FCOL", [36, RB], dt.float16, kind="ExternalInput")
    OUT = nc.dram_tensor("OUT", [C, nb * RB, xw], dt.float32, kind="ExternalOutput")

    n_grp = (nb + 6) // 7
    grp_rows = [min(7, nb - 7 * g) * RB for g in range(n_grp)]

    DErf = getattr(mybir.ActivationFunctionType, ACT_FUNC)

    with TileContext(nc) as tc:
        with (
            tc.tile_pool(name="singles", bufs=1) as psingle,
            tc.tile_pool(name="stx", bufs=2) as pstx,
            tc.tile_pool(name="cc", bufs=2) as pcc,
            tc.tile_pool(name="subeg", bufs=2) as psubeg,
            tc.tile_pool(name="e01", bufs=2) as pe01,
            tc.tile_pool(name="ft", bufs=4) as pft,
            tc.tile_pool(name="vv", bufs=3) as pvv,
            tc.tile_pool(name="psum", bufs=1, space="PSUM") as ppsum,
            tc.tile_pool(name="stage", bufs=2) as pstage,
            tc.tile_pool(name="gather", bufs=1) as pgather,
            tc.tile_pool(name="fin", bufs=1) as pfin,
        ):
            col = psingle.tile([P, K, RB], dt.float16, tag="col", name="col")
            nc.sync.dma_start(col[:, :, :], COL[:, :, :])
            pm = psingle.tile([P, 3, P], dt.float16, tag="pm", name="pm")
            nc.sync.dma_start(pm[:, :, :], PM[:, :, :])
            fcol = psingle.tile([36, RB], dt.float16, tag="fcol", name="fcol")
            nc.sync.dma_start(fcol[:, :], FCOL[:, :])
            frs = psingle.tile([36, C, ext], dt.float16, tag="frs", name="frs")
            nc.sync.dma_start(frs[:, :, :], FRS[:, :, :])
            frc = psingle.tile([36, C, ext], dt.float16, tag="frc", name="frc")
            nc.sync.dma_start(frc[:, :, :], FRC[:, :, :])

            # ---- fringe: direct D-path on 36 partitions
            fsub = psingle.tile([36, C, ext], dt.float16, tag="fsub", name="fsub")
            nc.vector.tensor_tensor(fsub[:, :, :], frs[:, :, :], frc[:, :, :],
                                    _alu("subtract"))
            feg = psingle.tile([36, C, ext], dt.float16, tag="feg", name="feg")
            nc.scalar.activation(feg[:, :, :], fsub[:, :, :], DErf,
                                 bias=0.0, scale=float(ACT_SCALE))
            fe01 = psingle.tile([36, ext], dt.float16, tag="fe01", name="fe01")
            nc.vector.tensor_tensor(fe01[:, :], feg[:, 0, :], feg[:, 1, :],
                                    _alu("mult"))
            fe = psingle.tile([36, ext], dt.float16, tag="fe", name="fe")
            nc.vector.tensor_tensor(fe[:, :], fe01[:, :], feg[:, 2, :],
                                    _alu("mult"))
            fv = psingle.tile([36, fw], dt.float16, tag="fv", name="fv")
            fpap = fv[:, 0].ap[0]
            fv_out = AP(fv.tensor, fv[:, 0].offset, [fpap, [xw, C], [1, xw]])
            fe_b = AP(fe.tensor, fe[:, 0].offset, [fe[:, 0].ap[0], [0, C], [1, xw]])
            fs_s = AP(frs.tensor, frs[:, 0, 0].offset,
                      [frs[:, 0, 0].ap[0], [ext, C], [1, xw]])
            nc.vector.tensor_tensor(fv_out, fe_b, fs_s, _alu("mult"))
            nc.vector.tensor_copy(fv[:, vgw:fw], fe[:, 0:xw])
            # bottom fringe needs its own base-partition-0 tiles for matmul
            fvb = psingle.tile([RB, fw], dt.float16, tag="fvb", name="fvb")
            nc.sync.dma_start(fvb[:, :], fv[RB:36, :])
            fcolb = psingle.tile([RB, RB], dt.float16, tag="fcolb", name="fcolb")
            nc.sync.dma_start(fcolb[:, :], fcol[RB:36, :])

            gat = {}
            for g in range(n_grp):
                gat[g] = pgather.tile([P, fw], dt.float16, tag=f"gat{g}",
                                      name=f"gat{g}")

            stx_t, cc_t, ft_t = {}, {}, {}
            pp_t = {}

            def load_dpath(b):
                stx = pstx.tile([P, C, sw], dt.float16, tag="stx", name="stx")
                nc.sync.dma_start(stx[:, :, :], STX[b])
                stx_t[b] = stx
                cc = pcc.tile([P, C, ext], dt.float16, tag="cc", name="cc")
                # center replicated over jy: src = STX[b, (3, r), c, 6+x]
                cbase = STX[b, 3 * RB, 0, 6]
                src = AP(
                    cbase.tensor, cbase.offset,
                    [[0, JY], [C * sw, RB], [sw, C], [1, ext]],
                )
                nc.sync.dma_start(cc[:, :, :], src)
                cc_t[b] = cc

                ft = pft.tile([P, K, ext], dt.float16, tag="ft", name="ft")
                ftp = ft[:, 0, 0].ap[0]
                pap = stx[:, 0, 0].ap[0]
                ccp = cc[:, 0, 0].ap[0]
                dsub = psubeg.tile([P, 4, C, ext], dt.float16, tag="dsub",
                                   name="dsub")
                s_ap = AP(stx.tensor, stx[:, 0, 3].offset,
                          [pap, [1, 4], [sw, C], [1, ext]])
                c_ap = AP(cc.tensor, cc[:, 0, 0].offset,
                          [ccp, [0, 4], [ext, C], [1, ext]])
                nc.vector.tensor_tensor(dsub[:, :, :, :], s_ap, c_ap,
                                        _alu("subtract"))
                eg = psubeg.tile([P, 4, C, ext], dt.float16, tag="eg",
                                 name="eg")
                nc.scalar.activation(eg[:, :, :, :], dsub[:, :, :, :], DErf,
                                     bias=0.0, scale=float(ACT_SCALE))
                egp = eg[:, 0, 0, 0].ap[0]
                e01 = pe01.tile([P, 4, ext], dt.float16, tag="e01", name="e01")
                nc.gpsimd.tensor_tensor(
                    e01[:, :, :],
                    AP(eg.tensor, eg[:, 0, 0, 0].offset,
                       [egp, [cg, 4], [1, ext]]),
                    AP(eg.tensor, eg[:, 0, 1, 0].offset,
                       [egp, [cg, 4], [1, ext]]),
                    _alu("mult"),
                )
                nc.vector.tensor_tensor(
                    AP(ft.tensor, ft[:, 0, 0].offset, [ftp, [ext, 4], [1, ext]]),
                    e01[:, :, :],
                    AP(eg.tensor, eg[:, 0, 2, 0].offset,
                       [egp, [cg, 4], [1, ext]]),
                    _alu("mult"),
                )
                ft_t[b] = ft

            def derive(j, ii):
                # derive group o' = 4+ii of block j via permute matmuls
                ft = ft_t[j]
                pm_order = [[0, 1, 2], [2, 1, 0], [0, 1, 2]]
                op_ = 4 + ii
                os_, sh_ = 6 - op_, op_ - 3
                pd = ppsum.tile([P, xw], dt.float32, tag="pd", name="pd")
                pres = [i for i in pm_order[ii] if 0 <= j + (i - 1) < nb]
                for k_i, i in enumerate(pres):
                    src_ft = ft_t[j + (i - 1)]
                    st_, sp_ = (k_i == 0), (k_i == len(pres) - 1)
                    for seg0, seg1 in ((0, MMN), (MMN, xw)):
                        nc.tensor.matmul(
                            pd[:, seg0:seg1],
                            pm[:, i, :],
                            src_ft[:, os_, sh_ + seg0: sh_ + seg1],
                            start=st_, stop=sp_,
                        )
                nc.scalar.copy(ft[:, op_, 0:xw], pd[:, :])

            def process(j):
                ft = ft_t[j]
                ftp = ft[:, 0, 0].ap[0]
                stx = stx_t[j]
                # pp first so it lands at psum offset 0 (bank-aligned segs)
                pp = ppsum.tile([RB, 3072], dt.float32, tag="pp", name="pp")
                pp_t[j] = pp
                edge = (j == 0) or (j == nb - 1)
                # psum layout: V [0,1920) banks 0-3, pad [1920,2048),
                # den [2048,2688) banks 4-5. src_a = col in v/fv coords.
                segs = [(0, 512, 0), (512, 1024, 512), (1024, 1536, 1024),
                        (1536, 1920, 1536), (2048, 2560, 1920),
                        (2560, 2688, 2432)]
                # interleave derivation of groups 4..6 between per-o V+MM
                # bundles so PE, ACT, and DVE all stay fed (HAM stays warm).
                for o in range(K):
                    if o < 3:
                        derive(j, o)
                    v = pvv.tile([P, C, xw], dt.float16, tag="v", name="v")
                    f_ap = AP(ft.tensor, ft[:, 0, 0].offset + o * ext,
                              [ftp, [0, C], [1, xw]])
                    s_ap = AP(stx.tensor, stx[:, 0, 3].offset + o,
                              [stx[:, 0, 0].ap[0], [sw, C], [1, xw]])
                    nc.vector.tensor_tensor(v[:, :, :], f_ap, s_ap,
                                            _alu("mult"))
                    for a, b_, sa in segs:
                        if a < vgw:
                            rhs = AP(v.tensor, v[:, 0, 0].offset + sa,
                                     [v[:, 0, 0].ap[0], [1, b_ - a]])
                        else:
                            rhs = ft[:, o, sa - vgw: sa - vgw + b_ - a]
                        nc.tensor.matmul(
                            pp[:, a:b_], col[:, o, :], rhs,
                            start=(o == 0),
                            stop=(o == K - 1 and not edge),
                        )
                if edge:
                    fc_, fv_ = (fcol, fv) if j == 0 else (fcolb, fvb)
                    for a, b_, sa in segs:
                        nc.tensor.matmul(
                            pp[:, a:b_], fc_[0:RB, :],
                            fv_[0:RB, sa: sa + b_ - a],
                            start=False, stop=True,
                        )

                # ---- evacuate + gather (stage packs [V | den] tightly)
                g, idx = j // 7, j % 7
                stg = pstage.tile([RB, fw], dt.float16, tag="stg", name="stg")
                nc.scalar.copy(stg[:, 0:vgw], pp[:, 0:vgw])
                nc.scalar.copy(stg[:, vgw:fw], pp[:, 2048:2688])
                rows = slice(idx * RB, (idx + 1) * RB)
                nc.sync.dma_start(gat[g][rows, :], stg[:, :])
                del pp_t[j], stx_t[j], cc_t[j]
                if j == 7 * g + 6 or j == nb - 1:
                    finalize(g)

            def finalize(g):
                rg = grp_rows[g]
                den32 = pfin.tile([P, xw], dt.float32, tag="den32", name="den32")
                nc.vector.tensor_copy(den32[0:rg, :], gat[g][0:rg, vgw:fw])
                rec32 = pfin.tile([P, xw], dt.float32, tag="rec32", name="rec32")
                nc.vector.reciprocal_approx_fast(rec32[0:rg, :], den32[0:rg, :])
                rec16 = pfin.tile([P, xw], dt.float16, tag="rec16", name="rec16")
                nc.vector.tensor_copy(rec16[0:rg, :], rec32[0:rg, :])
                for c in range(C):
                    ot = pfin.tile([P, xw], dt.float32, tag="ot", name="ot")
                    nc.gpsimd.tensor_tensor(
                        ot[0:rg, :], gat[g][0:rg, c * xw:(c + 1) * xw],
                        rec16[0:rg, :], _alu("mult"),
                    )
                    nc.sync.dma_start(OUT[c, g * P: g * P + rg, :], ot[0:rg, :])

            for b in range(nb):
                load_dpath(b)
                if b >= 1:
                    process(b - 1)
            process(nb - 1)

    nc.compile()
    return nc


def _gw7():
    ax = np.arange(-K // 2 + 1.0, K // 2 + 1.0)
    xx, yy = np.meshgrid(ax, ax)
    kern = np.exp(-(xx**2 + yy**2) / (2.0 * 5.0**2))
    return (kern / (1.0 / (2 * math.pi * 5.0**2))).astype(np.float64)


def host_prepare(I: np.ndarray, gw49: np.ndarray):
    """I: (1, C, Him, Wim) fp32 -> per-core input dicts."""
    _, c_, him, wim = I.shape
    nb = him // (HSH * RB)
    xw = wim // WSH
    sw = xw + 12
    rs = nb * RB

    gw7 = gw49.reshape(K, K).astype(np.float64)

    Ip = np.zeros((C, him + 2 * PAD, wim + 12), dtype=F32)
    Ip[:, PAD: PAD + him, 6: 6 + wim] = I[0]
    Ib = Ip.astype(F16)

    # collapse weights: COL[(jy,r), o, r] = gw7[jy, o]
    colw = np.zeros((P, K, RB), dtype=F16)
    for jy in range(JY):
        for r in range(RB):
            colw[jy * RB + r, :, r] = gw7[jy, :].astype(F16)

    # permute matrices PMm/PM0/PMp: PM[p_src, i, p']
    pmw = np.zeros((P, 3, P), dtype=F16)
    for jyp in range(JY):
        for r in range(RB):
            pp_ = jyp * RB + r
            rs_ = r + jyp - 3
            base = (6 - jyp) * RB
            if rs_ < 0:
                pmw[base + rs_ + RB, 0, pp_] = 1.0
            elif rs_ < RB:
                pmw[base + rs_, 1, pp_] = 1.0
            else:
                pmw[base + rs_ - RB, 2, pp_] = 1.0

    in_maps = []
    for i in range(N_CORES):
        hi, wi = i // WSH, i % WSH
        sh = Ib[:, rs * hi: rs * hi + rs + 2 * PAD, xw * wi: xw * wi + sw]
        s0, s1, s2 = sh.strides
        w1 = np.lib.stride_tricks.as_strided(
            sh, shape=(C, nb, JY, RB, sw),
            strides=(s0, RB * s1, s1, s1, s2),
        )
        STa = np.ascontiguousarray(
            w1.transpose(1, 2, 3, 0, 4)).reshape(nb, P, C, sw)

        shp = np.zeros((C, rs + 2 * PAD, sw + 3), dtype=F16)
        shp[:, :, :sw] = sh
        frs = np.zeros((36, C, EXT), dtype=F16)
        frc = np.zeros((36, C, EXT), dtype=F16)
        fcol = np.zeros((36, RB), dtype=F16)
        for t, (dx, (r, dy)) in enumerate(
            [(dx, rd) for dx in (1, 2, 3) for rd in _TOP_RD]
        ):
            frs[t] = shp[:, PAD + r + dy, 6 + dx: 6 + dx + EXT]
            frc[t] = shp[:, PAD + r, 6: 6 + EXT]
            fcol[t, r] = gw7[dy + 3, dx + 3]
        for t, (dx, (r, dy)) in enumerate(
            [(dx, rd) for dx in (1, 2, 3) for rd in _BOT_RD]
        ):
            rr = (nb - 1) * RB + r
            frs[18 + t] = shp[:, PAD + rr + dy, 6 + dx: 6 + dx + EXT]
            frc[18 + t] = shp[:, PAD + rr, 6: 6 + EXT]
            fcol[18 + t, r] = gw7[dy + 3, dx + 3]

        in_maps.append({
            "STX": STa, "COL": colw, "PM": pmw,
            "FRS": frs, "FRC": frc, "FCOL": fcol,
        })
    return in_maps, nb, xw, rs


def assemble(results, him, wim, rs, xw):
    out = np.empty((1, C, him, wim), dtype=F32)
    for i in range(N_CORES):
        hi, wi = i // WSH, i % WSH
        out[0, :, rs * hi: rs * hi + rs, xw * wi: xw * wi + xw] = \
            results[i]["OUT"]
    return out


def _numpy_fallback(I, g):
    n, c, h, w = I.shape
    Ipad = np.zeros((n, c, h + 2 * PAD, w + 2 * PAD), dtype=np.float64)
    Ipad[:, :, PAD: PAD + h, PAD: PAD + w] = I
    num = np.zeros((n, c, h, w), dtype=np.float64)
    den = np.zeros((n, h, w), dtype=np.float64)
    g64 = g.astype(np.float64)
    for j in range(K * K):
        dy, dx = j // K, j % K
        S = Ipad[:, :, dy: dy + h, dx: dx + w]
        D = ((S - I.astype(np.float64)) ** 2).sum(axis=1)
        wgt = np.exp(EXP_SCALE * D) * NORM_COLOR * g64[:, j]
        num += wgt[:, None] * S
        den += wgt
    return (num / den[:, None]).astype(F32)


_CACHE = {}
TRACE = False
LAST_EXEC_NS = None
_LDW_PATCHED = False


def _enable_ldw_prune():
    global _LDW_PATCHED
    if _LDW_PATCHED:
        return
    import json as _json
    import concourse.bass_utils as _bu

    _orig = _bu.compile_bir_kernel

    def _prune(bir_json):
        js = _json.loads(bir_json)
        for fn in js.get("functions", []):
            for blk in fn.get("blocks", []):
                insts = blk.get("instructions", [])
                out = []
                last_ldw = None
                for inst in insts:
                    if inst.get("opcode") == "Ldweights":
                        si = inst.get("sync_info") or {}
                        key = _json.dumps(inst.get("ins"), sort_keys=True)
                        if (last_ldw == key and not si.get("on_wait")
                                and not si.get("on_update")):
                            continue
                        last_ldw = key
                    out.append(inst)
                blk["instructions"] = out
        return _json.dumps(js).encode()

    def _patched(bir_json, tmpdir, neff_name="file.neff"):
        try:
            bir_json = _prune(bir_json)
        except Exception:
            pass
        return _orig(bir_json, tmpdir, neff_name=neff_name)

    _bu.compile_bir_kernel = _patched
    try:
        import concourse.bass2jax as _b2j
        if getattr(_b2j, "compile_bir_kernel", None) is not None:
            _b2j.compile_bir_kernel = _patched
    except Exception:
        pass
    _LDW_PATCHED = True


def kernel(I: np.ndarray, g: np.ndarray) -> np.ndarray:
    global LAST_EXEC_NS
    I = np.asarray(I, dtype=F32)
    g = np.asarray(g)

    gw49 = np.asarray(g[0, :, 0, 0], dtype=F32)
    if not np.array_equal(
        np.asarray(g), np.broadcast_to(np.asarray(g)[:, :, :1, :1], g.shape)
    ):
        return _numpy_fallback(I, g)

    from concourse.bass_utils import run_bass_kernel_spmd
    import os as _os
    if _os.environ.get("BASS_LDW_PRUNE", "1") == "1":
        _enable_ldw_prune()

    in_maps, nb, xw, rs = host_prepare(I, gw49)
    key = (nb, xw)
    if key not in _CACHE:
        _CACHE[key] = build_nc(nb, xw)
    nc = _CACHE[key]
    res = run_bass_kernel_spmd(
        nc, in_maps, core_ids=list(range(N_CORES)), trace=TRACE
    )
    LAST_EXEC_NS = res.exec_time_ns
    return assemble(res.results, I.shape[2], I.shape[3], rs, xw)


def _numpy_mirror_square(I, gw49):
    """Mirror of the device algorithm with Square standing in for the
    gaussian (for CoreSim validation of the AP/permute/fringe machinery)."""
    n, c, h, w = I.shape
    gw7 = gw49.reshape(K, K).astype(np.float64)
    Ipad = np.zeros((n, c, h + 2 * PAD, w + 2 * PAD), dtype=np.float64)
    Ipad[:, :, PAD: PAD + h, PAD: PAD + w] = I
    num = np.zeros((n, c, h, w), dtype=np.float64)
    den = np.zeros((n, h, w), dtype=np.float64)
    for j in range(K * K):
        dy, dx = j // K, j % K
        S = Ipad[:, :, dy: dy + h, dx: dx + w]
        diff = S - I.astype(np.float64)
        E = np.square(math.sqrt(0.5) * diff)
        wgt = E.prod(axis=1) * gw7[dy, dx]
        num += wgt[:, None] * S
        den += wgt
    return (num / den[:, None]).astype(F32)


if __name__ == "__main__":
    import concourse.bass_interp as bass_interp

    globals()["ACT_FUNC"] = "Square"
    globals()["ACT_SCALE"] = math.sqrt(0.5)

    rng = np.random.default_rng(0)
    him, wim = HSH * RB * 3, W  # 3 blocks per core
    I = rng.random((1, C, him, wim), dtype=F32)
    gw49 = _gw7().reshape(-1).astype(F32)

    in_maps, nb, xw, rs = host_prepare(I, gw49)
    nc = build_nc(nb, xw)
    sim = bass_interp.CoreSim(nc)
    for k, v in in_maps[0].items():
        sim.tensor(k)[:] = v
    sim.simulate()
    got = np.array(sim.tensor("OUT"))

    exp_full = _numpy_mirror_square(I, gw49)
    exp0 = exp_full[0, :, 0:rs, 0:xw]
    err = np.abs(got - exp0)
    print("sim err max:", err.max(), "rel:", err.max() / np.abs(exp0).max())
    # per-region check to localize issues
    for name, sl in [("top3rows", np.s_[:, 0:3, :]),
                     ("bot3rows", np.s_[:, rs - 3: rs, :]),
                     ("mid", np.s_[:, 3: rs - 3, :]),
                     ("blk-edge", np.s_[:, 16:20, :])]:
        e = np.abs(got[sl] - exp0[sl]).max()
        print(f"  {name:10s} max err {e:.5f}")


# revision 4
# speedup vs baseline: 1.1543x; 1.1543x over previous
"""Bilateral filter (7x7, sigma_color=0.1) Trainium2 Bass kernel, v6.

Key structure (per core; image sharded 4H x 2W over 8 cores):
  - STX[b, (jy,r), c, xe] fp16: x-extended strip stack, ONE copy (no 7x
    o-expansion). cc center tile loaded from STX via jy=3-replicated DMA.
  - D-path with offset symmetry E(p,d) = E(p+d,-d): only o in {0,1,2,3}
    computed directly; o in {4,5,6} derived by partition-permute matmuls
    (PMm/PM0/PMp for cross-block rows) + x-shifted column slice.
  - E = prod_c Derivative_Erf(sqrt(50)*diff_c): one ACT op replaces
    Square + channel adds + Exp. The (2/sqrt(pi))^3 factor cancels in
    num/den.
  - Spatial weights g folded into per-o collapse matmul weights (COL),
    so exp needs no bias and derived E tiles are directly reusable.
  - Boundary terms whose symmetric source lies outside the core's rows
    are computed directly on a 36-partition fringe tile and accumulated
    into the first/last block's psum.
  - psum [18, 2560] = [V0 V1 V2 den], fp16 evac, gather 7 blocks, then
    reciprocal_approx_fast + gpsimd mults.
"""

import math

import numpy as np

import concourse.bass as bass
import concourse.bacc as bacc
import concourse.mybir as mybir
from concourse.tile import TileContext
from concourse.bass import AP

F16 = np.float16
F32 = np.float32

H, W, C = 720, 1280, 3
K = 7
PAD = 3
SIGMA_COLOR = 0.1
EXP_SCALE = -1.0 / (2.0 * SIGMA_COLOR**2)  # -50.0
ERF_SCALE = math.sqrt(-EXP_SCALE)  # sqrt(50)
NORM_COLOR = 1.0 / (2.0 * math.pi * SIGMA_COLOR**2)

HSH, WSH = 4, 2
RB = 18
JY = 7
P = JY * RB  # 126
XW = W // WSH  # 640
EXT = 644    # E-tile group width (cols [0, 643) valid, 643 junk pad)
SW = 652     # stx width
N_CORES = 8
MMN = 512

# fringe combos: (r, dy) pairs whose symmetric source row is outside the core
_TOP_RD = [(0, -1), (0, -2), (0, -3), (1, -2), (1, -3), (2, -3)]
_BOT_RD = [(17, 1), (17, 2), (17, 3), (16, 2), (16, 3), (15, 3)]


def _alu(name):
    return getattr(mybir.AluOpType, name)


ACT_FUNC = "Derivative_Erf"  # smoke test overrides (must be an even fn)
ACT_SCALE = ERF_SCALE


def build_nc(nb: int, xw: int = XW):
    dt = mybir.dt
    nc = bacc.Bacc("TRN2", debug=False)
    ext = EXT
    sw = xw + 12
    cg = C * ext      # 1932
    fdsub = 4 * cg    # 7728
    vgw = C * xw      # 1920 packed V per o
    fw = vgw + xw     # 2560 psum width

    STX = nc.dram_tensor("STX", [nb, P, C, sw], dt.float16, kind="ExternalInput")
    COL = nc.dram_tensor("COL", [P, K, RB], dt.float16, kind="ExternalInput")
    PM = nc.dram_tensor("PM", [P, 3, P], dt.float16, kind="ExternalInput")
    FRS = nc.dram_tensor("FRS", [36, C, ext], dt.float16, kind="ExternalInput")
    FRC = nc.dram_tensor("FRC", [36, C, ext], dt.float16, kind="ExternalInput")
    FCOL = nc.dram_tensor("FCOL", [36, RB], dt.float16, kind="ExternalInput")
    OUT = nc.dram_tensor("OUT", [C, nb * RB, xw], dt.float32, kind="ExternalOutput")

    n_grp = (nb + 6) // 7
    grp_rows = [min(7, nb - 7 * g) * RB for g in range(n_grp)]

    DErf = getattr(mybir.ActivationFunctionType, ACT_FUNC)

    with TileContext(nc) as tc:
        with (
            tc.tile_pool(name="singles", bufs=1) as psingle,
            tc.tile_pool(name="stx", bufs=3) as pstx,
            tc.tile_pool(name="cc", bufs=2) as pcc,
            tc.tile_pool(name="subeg", bufs=2) as psubeg,
            tc.tile_pool(name="e01", bufs=2) as pe01,
            tc.tile_pool(name="ft", bufs=4) as pft,
            tc.tile_pool(name="vv", bufs=3) as pvv,
            tc.tile_pool(name="psum", bufs=1, space="PSUM") as ppsum,
            tc.tile_pool(name="stage", bufs=2) as pstage,
            tc.tile_pool(name="gather", bufs=1) as pgather,
            tc.tile_pool(name="fin", bufs=1) as pfin,
        ):
            col = psingle.tile([P, K, RB], dt.float16, tag="col", name="col")
            nc.sync.dma_start(col[:, :, :], COL[:, :, :])
            pm = psingle.tile([P, 3, P], dt.float16, tag="pm", name="pm")
            nc.sync.dma_start(pm[:, :, :], PM[:, :, :])
            fcol = psingle.tile([36, RB], dt.float16, tag="fcol", name="fcol")
            nc.sync.dma_start(fcol[:, :], FCOL[:, :])
            frs = psingle.tile([36, C, ext], dt.float16, tag="frs", name="frs")
            nc.sync.dma_start(frs[:, :, :], FRS[:, :, :])
            frc = psingle.tile([36, C, ext], dt.float16, tag="frc", name="frc")
            nc.sync.dma_start(frc[:, :, :], FRC[:, :, :])

            # ---- fringe: direct D-path on 36 partitions
            fsub = psingle.tile([36, C, ext], dt.float16, tag="fsub", name="fsub")
            nc.vector.tensor_tensor(fsub[:, :, :], frs[:, :, :], frc[:, :, :],
                                    _alu("subtract"))
            feg = psingle.tile([36, C, ext], dt.float16, tag="feg", name="feg")
            nc.scalar.activation(feg[:, :, :], fsub[:, :, :], DErf,
                                 bias=0.0, scale=float(ACT_SCALE))
            fe01 = psingle.tile([36, ext], dt.float16, tag="fe01", name="fe01")
            nc.vector.tensor_tensor(fe01[:, :], feg[:, 0, :], feg[:, 1, :],
                                    _alu("mult"))
            fe = psingle.tile([36, ext], dt.float16, tag="fe", name="fe")
            nc.vector.tensor_tensor(fe[:, :], fe01[:, :], feg[:, 2, :],
                                    _alu("mult"))
            fv = psingle.tile([36, fw], dt.float16, tag="fv", name="fv")
            fpap = fv[:, 0].ap[0]
            fv_out = AP(fv.tensor, fv[:, 0].offset, [fpap, [xw, C], [1, xw]])
            fe_b = AP(fe.tensor, fe[:, 0].offset, [fe[:, 0].ap[0], [0, C], [1, xw]])
            fs_s = AP(frs.tensor, frs[:, 0, 0].offset,
                      [frs[:, 0, 0].ap[0], [ext, C], [1, xw]])
            nc.vector.tensor_tensor(fv_out, fe_b, fs_s, _alu("mult"))
            nc.vector.tensor_copy(fv[:, vgw:fw], fe[:, 0:xw])
            # bottom fringe needs its own base-partition-0 tiles for matmul
            fvb = psingle.tile([RB, fw], dt.float16, tag="fvb", name="fvb")
            nc.sync.dma_start(fvb[:, :], fv[RB:36, :])
            fcolb = psingle.tile([RB, RB], dt.float16, tag="fcolb", name="fcolb")
            nc.sync.dma_start(fcolb[:, :], fcol[RB:36, :])

            gat = {}
            for g in range(n_grp):
                gat[g] = pgather.tile([P, fw], dt.float16, tag=f"gat{g}",
                                      name=f"gat{g}")

            stx_t, cc_t, ft_t, eg_t = {}, {}, {}, {}
            pp_t = {}

            def load_dpath(b):
                stx = pstx.tile([P, C, sw], dt.float16, tag="stx", name="stx")
                nc.sync.dma_start(stx[:, :, :], STX[b])
                stx_t[b] = stx
                cc = pcc.tile([P, C, ext], dt.float16, tag="cc", name="cc")
                # center replicated over jy: src = STX[b, (3, r), c, 6+x]
                cbase = STX[b, 3 * RB, 0, 6]
                src = AP(
                    cbase.tensor, cbase.offset,
                    [[0, JY], [C * sw, RB], [sw, C], [1, ext]],
                )
                nc.sync.dma_start(cc[:, :, :], src)
                cc_t[b] = cc

                ft = pft.tile([P, K, ext], dt.float16, tag="ft", name="ft")
                ftp = ft[:, 0, 0].ap[0]
                pap = stx[:, 0, 0].ap[0]
                ccp = cc[:, 0, 0].ap[0]
                dsub = psubeg.tile([P, 4, C, ext], dt.float16, tag="dsub",
                                   name="dsub")
                s_ap = AP(stx.tensor, stx[:, 0, 3].offset,
                          [pap, [1, 4], [sw, C], [1, ext]])
                c_ap = AP(cc.tensor, cc[:, 0, 0].offset,
                          [ccp, [0, 4], [ext, C], [1, ext]])
                nc.vector.tensor_tensor(dsub[:, :, :, :], s_ap, c_ap,
                                        _alu("subtract"))
                eg = psubeg.tile([P, 4, C, ext], dt.float16, tag="eg",
                                 name="eg")
                nc.scalar.activation(eg[:, :, :, :], dsub[:, :, :, :], DErf,
                                     bias=0.0, scale=float(ACT_SCALE))
                egp = eg[:, 0, 0, 0].ap[0]
                e01 = pe01.tile([P, 4, ext], dt.float16, tag="e01", name="e01")
                nc.gpsimd.tensor_tensor(
                    e01[:, :, :],
                    AP(eg.tensor, eg[:, 0, 0, 0].offset,
                       [egp, [cg, 4], [1, ext]]),
                    AP(eg.tensor, eg[:, 0, 1, 0].offset,
                       [egp, [cg, 4], [1, ext]]),
                    _alu("mult"),
                )
                ft_t[b] = ft
                eg_t[b] = (eg, e01)

            def dpath2(b):
                ft = ft_t[b]
                ftp = ft[:, 0, 0].ap[0]
                eg, e01 = eg_t[b]
                egp = eg[:, 0, 0, 0].ap[0]
                nc.vector.tensor_tensor(
                    AP(ft.tensor, ft[:, 0, 0].offset, [ftp, [ext, 4], [1, ext]]),
                    e01[:, :, :],
                    AP(eg.tensor, eg[:, 0, 2, 0].offset,
                       [egp, [cg, 4], [1, ext]]),
                    _alu("mult"),
                )
                del eg_t[b]

            def derive(j, ii):
                # derive group o' = 4+ii of block j via permute matmuls
                ft = ft_t[j]
                pm_order = [[0, 1, 2], [2, 1, 0], [0, 1, 2]]
                op_ = 4 + ii
                os_, sh_ = 6 - op_, op_ - 3
                pd = ppsum.tile([P, xw], dt.float32, tag="pd", name="pd")
                pres = [i for i in pm_order[ii] if 0 <= j + (i - 1) < nb]
                for k_i, i in enumerate(pres):
                    src_ft = ft_t[j + (i - 1)]
                    st_, sp_ = (k_i == 0), (k_i == len(pres) - 1)
                    for seg0, seg1 in ((0, MMN), (MMN, xw)):
                        nc.tensor.matmul(
                            pd[:, seg0:seg1],
                            pm[:, i, :],
                            src_ft[:, os_, sh_ + seg0: sh_ + seg1],
                            start=st_, stop=sp_,
                        )
                nc.scalar.copy(ft[:, op_, 0:xw], pd[:, :])

            def process(j):
                ft = ft_t[j]
                ftp = ft[:, 0, 0].ap[0]
                stx = stx_t[j]
                # pp first so it lands at psum offset 0 (bank-aligned segs)
                pp = ppsum.tile([RB, 3072], dt.float32, tag="pp", name="pp")
                pp_t[j] = pp
                edge = (j == 0) or (j == nb - 1)
                # psum layout: V [0,1920) banks 0-3, pad [1920,2048),
                # den [2048,2688) banks 4-5. src_a = col in v/fv coords.
                segs = [(0, 512, 0), (512, 1024, 512), (1024, 1536, 1024),
                        (1536, 1920, 1536), (2048, 2560, 1920),
                        (2560, 2688, 2432)]
                # V-bundles for source groups 0..3 (no cross-block deps):
                # these fill the DVE while block j+1's EG/e01 chain runs.
                for o in range(4):
                    vbundle(j, o, pp, segs, edge)

            def vbundle(j, o, pp, segs, edge):
                ft = ft_t[j]
                ftp = ft[:, 0, 0].ap[0]
                stx = stx_t[j]
                v = pvv.tile([P, C, xw], dt.float16, tag="v", name="v")
                f_ap = AP(ft.tensor, ft[:, 0, 0].offset + o * ext,
                          [ftp, [0, C], [1, xw]])
                s_ap = AP(stx.tensor, stx[:, 0, 3].offset + o,
                          [stx[:, 0, 0].ap[0], [sw, C], [1, xw]])
                nc.vector.tensor_tensor(v[:, :, :], f_ap, s_ap, _alu("mult"))
                for a, b_, sa in segs:
                    if a < vgw:
                        rhs = AP(v.tensor, v[:, 0, 0].offset + sa,
                                 [v[:, 0, 0].ap[0], [1, b_ - a]])
                    else:
                        rhs = ft[:, o, sa - vgw: sa - vgw + b_ - a]
                    nc.tensor.matmul(
                        pp[:, a:b_], col[:, o, :], rhs,
                        start=(o == 0),
                        stop=(o == K - 1 and not edge),
                    )

            def process2(j):
                ft = ft_t[j]
                pp = pp_t[j]
                edge = (j == 0) or (j == nb - 1)
                segs = [(0, 512, 0), (512, 1024, 512), (1024, 1536, 1024),
                        (1536, 1920, 1536), (2048, 2560, 1920),
                        (2560, 2688, 2432)]
                for o in range(4, K):
                    derive(j, o - 4)
                    vbundle(j, o, pp, segs, edge)
                if edge:
                    fc_, fv_ = (fcol, fv) if j == 0 else (fcolb, fvb)
                    for a, b_, sa in segs:
                        nc.tensor.matmul(
                            pp[:, a:b_], fc_[0:RB, :],
                            fv_[0:RB, sa: sa + b_ - a],
                            start=False, stop=True,
                        )

                # ---- evacuate + gather (stage packs [V | den] tightly)
                g, idx = j // 7, j % 7
                stg = pstage.tile([RB, fw], dt.float16, tag="stg", name="stg")
                nc.scalar.copy(stg[:, 0:vgw], pp[:, 0:vgw])
                nc.scalar.copy(stg[:, vgw:fw], pp[:, 2048:2688])
                rows = slice(idx * RB, (idx + 1) * RB)
                nc.sync.dma_start(gat[g][rows, :], stg[:, :])
                del pp_t[j], stx_t[j], cc_t[j]
                if j == 7 * g + 6 or j == nb - 1:
                    finalize(g)

            def finalize(g):
                rg = grp_rows[g]
                den32 = pfin.tile([P, xw], dt.float32, tag="den32", name="den32")
                nc.vector.tensor_copy(den32[0:rg, :], gat[g][0:rg, vgw:fw])
                rec32 = pfin.tile([P, xw], dt.float32, tag="rec32", name="rec32")
                nc.vector.reciprocal_approx_fast(rec32[0:rg, :], den32[0:rg, :])
                rec16 = pfin.tile([P, xw], dt.float16, tag="rec16", name="rec16")
                nc.vector.tensor_copy(rec16[0:rg, :], rec32[0:rg, :])
                for c in range(C):
                    ot = pfin.tile([P, xw], dt.float32, tag="ot", name="ot")
                    nc.gpsimd.tensor_tensor(
                        ot[0:rg, :], gat[g][0:rg, c * xw:(c + 1) * xw],
                        rec16[0:rg, :], _alu("mult"),
                    )
                    nc.sync.dma_start(OUT[c, g * P: g * P + rg, :], ot[0:rg, :])

            for b in range(nb):
                load_dpath(b)
                if b >= 1:
                    process(b - 1)
                dpath2(b)
                if b >= 1:
                    process2(b - 1)
            process(nb - 1)
            process2(nb - 1)

    nc.compile()
    return nc


def _gw7():
    ax = np.arange(-K // 2 + 1.0, K // 2 + 1.0)
    xx, yy = np.meshgrid(ax, ax)
    kern = np.exp(-(xx**2 + yy**2) / (2.0 * 5.0**2))
    return (kern / (1.0 / (2 * math.pi * 5.0**2))).astype(np.float64)


def host_prepare(I: np.ndarray, gw49: np.ndarray):
    """I: (1, C, Him, Wim) fp32 -> per-core input dicts."""
    _, c_, him, wim = I.shape
    nb = him // (HSH * RB)
    xw = wim // WSH
    sw = xw + 12
    rs = nb * RB

    gw7 = gw49.reshape(K, K).astype(np.float64)

    Ip = np.zeros((C, him + 2 * PAD, wim + 12), dtype=F32)
    Ip[:, PAD: PAD + him, 6: 6 + wim] = I[0]
    Ib = Ip.astype(F16)

    # collapse weights: COL[(jy,r), o, r] = gw7[jy, o]
    colw = np.zeros((P, K, RB), dtype=F16)
    for jy in range(JY):
        for r in range(RB):
            colw[jy * RB + r, :, r] = gw7[jy, :].astype(F16)

    # permute matrices PMm/PM0/PMp: PM[p_src, i, p']
    pmw = np.zeros((P, 3, P), dtype=F16)
    for jyp in range(JY):
        for r in range(RB):
            pp_ = jyp * RB + r
            rs_ = r + jyp - 3
            base = (6 - jyp) * RB
            if rs_ < 0:
                pmw[base + rs_ + RB, 0, pp_] = 1.0
            elif rs_ < RB:
                pmw[base + rs_, 1, pp_] = 1.0
            else:
                pmw[base + rs_ - RB, 2, pp_] = 1.0

    in_maps = []
    for i in range(N_CORES):
        hi, wi = i // WSH, i % WSH
        sh = Ib[:, rs * hi: rs * hi + rs + 2 * PAD, xw * wi: xw * wi + sw]
        s0, s1, s2 = sh.strides
        w1 = np.lib.stride_tricks.as_strided(
            sh, shape=(C, nb, JY, RB, sw),
            strides=(s0, RB * s1, s1, s1, s2),
        )
        STa = np.ascontiguousarray(
            w1.transpose(1, 2, 3, 0, 4)).reshape(nb, P, C, sw)

        shp = np.zeros((C, rs + 2 * PAD, sw + 3), dtype=F16)
        shp[:, :, :sw] = sh
        frs = np.zeros((36, C, EXT), dtype=F16)
        frc = np.zeros((36, C, EXT), dtype=F16)
        fcol = np.zeros((36, RB), dtype=F16)
        for t, (dx, (r, dy)) in enumerate(
            [(dx, rd) for dx in (1, 2, 3) for rd in _TOP_RD]
        ):
            frs[t] = shp[:, PAD + r + dy, 6 + dx: 6 + dx + EXT]
            frc[t] = shp[:, PAD + r, 6: 6 + EXT]
            fcol[t, r] = gw7[dy + 3, dx + 3]
        for t, (dx, (r, dy)) in enumerate(
            [(dx, rd) for dx in (1, 2, 3) for rd in _BOT_RD]
        ):
            rr = (nb - 1) * RB + r
            frs[18 + t] = shp[:, PAD + rr + dy, 6 + dx: 6 + dx + EXT]
            frc[18 + t] = shp[:, PAD + rr, 6: 6 + EXT]
            fcol[18 + t, r] = gw7[dy + 3, dx + 3]

        in_maps.append({
            "STX": STa, "COL": colw, "PM": pmw,
            "FRS": frs, "FRC": frc, "FCOL": fcol,
        })
    return in_maps, nb, xw, rs


def assemble(results, him, wim, rs, xw):
    out = np.empty((1, C, him, wim), dtype=F32)
    for i in range(N_CORES):
        hi, wi = i // WSH, i % WSH
        out[0, :, rs * hi: rs * hi + rs, xw * wi: xw * wi + xw] = \
            results[i]["OUT"]
    return out


def _numpy_fallback(I, g):
    n, c, h, w = I.shape
    Ipad = np.zeros((n, c, h + 2 * PAD, w + 2 * PAD), dtype=np.float64)
    Ipad[:, :, PAD: PAD + h, PAD: PAD + w] = I
    num = np.zeros((n, c, h, w), dtype=np.float64)
    den = np.zeros((n, h, w), dtype=np.float64)
    g64 = g.astype(np.float64)
    for j in range(K * K):
        dy, dx = j // K, j % K
        S = Ipad[:, :, dy: dy + h, dx: dx + w]
        D = ((S - I.astype(np.float64)) ** 2).sum(axis=1)
        wgt = np.exp(EXP_SCALE * D) * NORM_COLOR * g64[:, j]
        num += wgt[:, None] * S
        den += wgt
    return (num / den[:, None]).astype(F32)


_CACHE = {}
TRACE = False
LAST_EXEC_NS = None
_LDW_PATCHED = False


def _enable_ldw_prune():
    global _LDW_PATCHED
    if _LDW_PATCHED:
        return
    import json as _json
    import concourse.bass_utils as _bu

    _orig = _bu.compile_bir_kernel

    def _prune(bir_json):
        js = _json.loads(bir_json)
        for fn in js.get("functions", []):
            for blk in fn.get("blocks", []):
                insts = blk.get("instructions", [])
                out = []
                last_ldw = None
                for inst in insts:
                    if inst.get("opcode") == "Ldweights":
                        si = inst.get("sync_info") or {}
                        key = _json.dumps(inst.get("ins"), sort_keys=True)
                        if (last_ldw == key and not si.get("on_wait")
                                and not si.get("on_update")):
                            continue
                        last_ldw = key
                    out.append(inst)
                blk["instructions"] = out
        return _json.dumps(js).encode()

    def _patched(bir_json, tmpdir, neff_name="file.neff"):
        try:
            bir_json = _prune(bir_json)
        except Exception:
            pass
        return _orig(bir_json, tmpdir, neff_name=neff_name)

    _bu.compile_bir_kernel = _patched
    try:
        import concourse.bass2jax as _b2j
        if getattr(_b2j, "compile_bir_kernel", None) is not None:
            _b2j.compile_bir_kernel = _patched
    except Exception:
        pass
    _LDW_PATCHED = True


def kernel(I: np.ndarray, g: np.ndarray) -> np.ndarray:
    global LAST_EXEC_NS
    I = np.asarray(I, dtype=F32)
    g = np.asarray(g)

    gw49 = np.asarray(g[0, :, 0, 0], dtype=F32)
    if not np.array_equal(
        np.asarray(g), np.broadcast_to(np.asarray(g)[:, :, :1, :1], g.shape)
    ):
        return _numpy_fallback(I, g)

    from concourse.bass_utils import run_bass_kernel_spmd
    import os as _os
    if _os.environ.get("BASS_LDW_PRUNE", "1") == "1":
        _enable_ldw_prune()

    in_maps, nb, xw, rs = host_prepare(I, gw49)
    key = (nb, xw)
    if key not in _CACHE:
        _CACHE[key] = build_nc(nb, xw)
    nc = _CACHE[key]
    res = run_bass_kernel_spmd(
        nc, in_maps, core_ids=list(range(N_CORES)), trace=TRACE
    )
    LAST_EXEC_NS = res.exec_time_ns
    return assemble(res.results, I.shape[2], I.shape[3], rs, xw)


def _numpy_mirror_square(I, gw49):
    """Mirror of the device algorithm with Square standing in for the
    gaussian (for CoreSim validation of the AP/permute/fringe machinery)."""
    n, c, h, w = I.shape
    gw7 = gw49.reshape(K, K).astype(np.float64)
    Ipad = np.zeros((n, c, h + 2 * PAD, w + 2 * PAD), dtype=np.float64)
    Ipad[:, :, PAD: PAD + h, PAD: PAD + w] = I
    num = np.zeros((n, c, h, w), dtype=np.float64)
    den = np.zeros((n, h, w), dtype=np.float64)
    for j in range(K * K):
        dy, dx = j // K, j % K
        S = Ipad[:, :, dy: dy + h, dx: dx + w]
        diff = S - I.astype(np.float64)
        E = np.square(math.sqrt(0.5) * diff)
        wgt = E.prod(axis=1) * gw7[dy, dx]
        num += wgt[:, None] * S
        den += wgt
    return (num / den[:, None]).astype(F32)


if __name__ == "__main__":
    import concourse.bass_interp as bass_interp

    globals()["ACT_FUNC"] = "Square"
    globals()["ACT_SCALE"] = math.sqrt(0.5)

    rng = np.random.default_rng(0)
    him, wim = HSH * RB * 3, W  # 3 blocks per core
    I = rng.random((1, C, him, wim), dtype=F32)
    gw49 = _gw7().reshape(-1).astype(F32)

    in_maps, nb, xw, rs = host_prepare(I, gw49)
    nc = build_nc(nb, xw)
    sim = bass_interp.CoreSim(nc)
    for k, v in in_maps[0].items():
        sim.tensor(k)[:] = v
    sim.simulate()
    got = np.array(sim.tensor("OUT"))

    exp_full = _numpy_mirror_square(I, gw49)
    exp0 = exp_full[0, :, 0:rs, 0:xw]
    err = np.abs(got - exp0)
    print("sim err max:", err.max(), "rel:", err.max() / np.abs(exp0).max())
    # per-region check to localize issues
    for name, sl in [("top3rows", np.s_[:, 0:3, :]),
                     ("bot3rows", np.s_[:, rs - 3: rs, :]),
                     ("mid", np.s_[:, 3: rs - 3, :]),
                     ("blk-edge", np.s_[:, 16:20, :])]:
        e = np.abs(got[sl] - exp0[sl]).max()
        print(f"  {name:10s} max err {e:.5f}")


# revision 5
# speedup vs baseline: 1.3107x; 1.1356x over previous
"""Bilateral filter (7x7, sigma_color=0.1) Trainium2 Bass kernel, v6.

Key structure (per core; image sharded 4H x 2W over 8 cores):
  - STX[b, (jy,r), c, xe] fp16: x-extended strip stack, ONE copy (no 7x
    o-expansion). cc center tile loaded from STX via jy=3-replicated DMA.
  - D-path with offset symmetry E(p,d) = E(p+d,-d): only o in {0,1,2,3}
    computed directly; o in {4,5,6} derived by partition-permute matmuls
    (PMm/PM0/PMp for cross-block rows) + x-shifted column slice.
  - E = prod_c Derivative_Erf(sqrt(50)*diff_c): one ACT op replaces
    Square + channel adds + Exp. The (2/sqrt(pi))^3 factor cancels in
    num/den.
  - Spatial weights g folded into per-o collapse matmul weights (COL),
    so exp needs no bias and derived E tiles are directly reusable.
  - Boundary terms whose symmetric source lies outside the core's rows
    are computed directly on a 36-partition fringe tile and accumulated
    into the first/last block's psum.
  - psum [18, 2560] = [V0 V1 V2 den], fp16 evac, gather 7 blocks, then
    reciprocal_approx_fast + gpsimd mults.
"""

import math

import numpy as np

import concourse.bass as bass
import concourse.bacc as bacc
import concourse.mybir as mybir
from concourse.tile import TileContext
from concourse.bass import AP

F16 = np.float16
F32 = np.float32

H, W, C = 720, 1280, 3
K = 7
PAD = 3
SIGMA_COLOR = 0.1
EXP_SCALE = -1.0 / (2.0 * SIGMA_COLOR**2)  # -50.0
ERF_SCALE = math.sqrt(-EXP_SCALE)  # sqrt(50)
NORM_COLOR = 1.0 / (2.0 * math.pi * SIGMA_COLOR**2)

HSH, WSH = 4, 2
RB = 18
JY = 7
P = JY * RB  # 126
XW = W // WSH  # 640
EXT = 644    # E-tile group width (cols [0, 643) valid, 643 junk pad)
SW = 652     # stx width
N_CORES = 8
MMN = 512

# fringe combos: (r, dy) pairs whose symmetric source row is outside the core
_TOP_RD = [(0, -1), (0, -2), (0, -3), (1, -2), (1, -3), (2, -3)]
_BOT_RD = [(17, 1), (17, 2), (17, 3), (16, 2), (16, 3), (15, 3)]


def _alu(name):
    return getattr(mybir.AluOpType, name)


ACT_FUNC = "Derivative_Erf"  # smoke test overrides (must be an even fn)
ACT_SCALE = ERF_SCALE


def build_nc(nb: int, xw: int = XW):
    dt = mybir.dt
    nc = bacc.Bacc("TRN2", debug=False)
    ext = EXT
    sw = xw + 12
    cg = C * ext      # 1932
    fdsub = 4 * cg    # 7728
    vgw = C * xw      # 1920 packed V per o
    fw = vgw + xw     # 2560 psum width

    STX = nc.dram_tensor("STX", [nb, P, C, sw], dt.float16, kind="ExternalInput")
    COL = nc.dram_tensor("COL", [P, K, RB], dt.float16, kind="ExternalInput")
    PM = nc.dram_tensor("PM", [P, 3, P], dt.float16, kind="ExternalInput")
    FRS = nc.dram_tensor("FRS", [36, C, ext], dt.float16, kind="ExternalInput")
    FRC = nc.dram_tensor("FRC", [36, C, ext], dt.float16, kind="ExternalInput")
    FCOL = nc.dram_tensor("FCOL", [36, RB], dt.float16, kind="ExternalInput")
    OUT = nc.dram_tensor("OUT", [C, nb * RB, xw], dt.float32, kind="ExternalOutput")

    n_grp = (nb + 6) // 7
    grp_rows = [min(7, nb - 7 * g) * RB for g in range(n_grp)]

    DErf = getattr(mybir.ActivationFunctionType, ACT_FUNC)

    with TileContext(nc) as tc:
        with (
            tc.tile_pool(name="singles", bufs=1) as psingle,
            tc.tile_pool(name="stx", bufs=3) as pstx,
            tc.tile_pool(name="cc", bufs=2) as pcc,
            tc.tile_pool(name="subeg", bufs=2) as psubeg,
            tc.tile_pool(name="e01", bufs=2) as pe01,
            tc.tile_pool(name="ft", bufs=4) as pft,
            tc.tile_pool(name="vv", bufs=3) as pvv,
            tc.tile_pool(name="psum", bufs=1, space="PSUM") as ppsum,
            tc.tile_pool(name="stage", bufs=2) as pstage,
            tc.tile_pool(name="gather", bufs=1) as pgather,
            tc.tile_pool(name="fin", bufs=1) as pfin,
        ):
            col = psingle.tile([P, K, RB], dt.float16, tag="col", name="col")
            nc.sync.dma_start(col[:, :, :], COL[:, :, :])
            pm = psingle.tile([P, 3, P], dt.float16, tag="pm", name="pm")
            nc.sync.dma_start(pm[:, :, :], PM[:, :, :])
            fcol = psingle.tile([36, RB], dt.float16, tag="fcol", name="fcol")
            nc.sync.dma_start(fcol[:, :], FCOL[:, :])
            frs = psingle.tile([36, C, ext], dt.float16, tag="frs", name="frs")
            nc.sync.dma_start(frs[:, :, :], FRS[:, :, :])
            frc = psingle.tile([36, C, ext], dt.float16, tag="frc", name="frc")
            nc.sync.dma_start(frc[:, :, :], FRC[:, :, :])

            # ---- fringe: direct D-path on 36 partitions
            fsub = psingle.tile([36, C, ext], dt.float16, tag="fsub", name="fsub")
            nc.vector.tensor_tensor(fsub[:, :, :], frs[:, :, :], frc[:, :, :],
                                    _alu("subtract"))
            feg = psingle.tile([36, C, ext], dt.float16, tag="feg", name="feg")
            nc.scalar.activation(feg[:, :, :], fsub[:, :, :], DErf,
                                 bias=0.0, scale=float(ACT_SCALE))
            fe01 = psingle.tile([36, ext], dt.float16, tag="fe01", name="fe01")
            nc.vector.tensor_tensor(fe01[:, :], feg[:, 0, :], feg[:, 1, :],
                                    _alu("mult"))
            fe = psingle.tile([36, ext], dt.float16, tag="fe", name="fe")
            nc.vector.tensor_tensor(fe[:, :], fe01[:, :], feg[:, 2, :],
                                    _alu("mult"))
            fv = psingle.tile([36, fw], dt.float16, tag="fv", name="fv")
            fpap = fv[:, 0].ap[0]
            fv_out = AP(fv.tensor, fv[:, 0].offset, [fpap, [xw, C], [1, xw]])
            fe_b = AP(fe.tensor, fe[:, 0].offset, [fe[:, 0].ap[0], [0, C], [1, xw]])
            fs_s = AP(frs.tensor, frs[:, 0, 0].offset,
                      [frs[:, 0, 0].ap[0], [ext, C], [1, xw]])
            nc.vector.tensor_tensor(fv_out, fe_b, fs_s, _alu("mult"))
            nc.vector.tensor_copy(fv[:, vgw:fw], fe[:, 0:xw])
            # bottom fringe needs its own base-partition-0 tiles for matmul
            fvb = psingle.tile([RB, fw], dt.float16, tag="fvb", name="fvb")
            nc.sync.dma_start(fvb[:, :], fv[RB:36, :])
            fcolb = psingle.tile([RB, RB], dt.float16, tag="fcolb", name="fcolb")
            nc.sync.dma_start(fcolb[:, :], fcol[RB:36, :])

            gat = {}
            for g in range(n_grp):
                gat[g] = pgather.tile([P, fw], dt.float16, tag=f"gat{g}",
                                      name=f"gat{g}")

            stx_t, cc_t, ft_t, eg_t = {}, {}, {}, {}
            pp_t = {}

            def load_dpath(b):
                stx = pstx.tile([P, C, sw], dt.float16, tag="stx", name="stx")
                nc.sync.dma_start(stx[:, :, :], STX[b])
                stx_t[b] = stx
                cc = pcc.tile([P, C, ext], dt.float16, tag="cc", name="cc")
                # center replicated over jy: src = STX[b, (3, r), c, 6+x]
                cbase = STX[b, 3 * RB, 0, 6]
                src = AP(
                    cbase.tensor, cbase.offset,
                    [[0, JY], [C * sw, RB], [sw, C], [1, ext]],
                )
                nc.sync.dma_start(cc[:, :, :], src)
                cc_t[b] = cc

                ft = pft.tile([P, K, ext], dt.float16, tag="ft", name="ft")
                ft_t[b] = ft
                pap = stx[:, 0, 0].ap[0]
                ccp = cc[:, 0, 0].ap[0]
                # halves: A = o in {2,3} (first: feeds derive o'=4 early),
                #         B = o in {0,1}
                for tag, obase in (("A", 2), ("B", 0)):
                    dsub = psubeg.tile([P, 2, C, ext], dt.float16,
                                       tag=f"dsub{tag}", name=f"dsub{tag}")
                    s_ap = AP(stx.tensor, stx[:, 0, 3].offset + obase,
                              [pap, [1, 2], [sw, C], [1, ext]])
                    c_ap = AP(cc.tensor, cc[:, 0, 0].offset,
                              [ccp, [0, 2], [ext, C], [1, ext]])
                    nc.vector.tensor_tensor(dsub[:, :, :, :], s_ap, c_ap,
                                            _alu("subtract"))
                    eg = psubeg.tile([P, 2, C, ext], dt.float16,
                                     tag=f"eg{tag}", name=f"eg{tag}")
                    nc.scalar.activation(eg[:, :, :, :], dsub[:, :, :, :],
                                         DErf, bias=0.0,
                                         scale=float(ACT_SCALE))
                    egp = eg[:, 0, 0, 0].ap[0]
                    e01 = pe01.tile([P, 2, ext], dt.float16,
                                    tag=f"e01{tag}", name=f"e01{tag}")
                    nc.gpsimd.tensor_tensor(
                        e01[:, :, :],
                        AP(eg.tensor, eg[:, 0, 0, 0].offset,
                           [egp, [cg, 2], [1, ext]]),
                        AP(eg.tensor, eg[:, 0, 1, 0].offset,
                           [egp, [cg, 2], [1, ext]]),
                        _alu("mult"),
                    )
                    eg_t[(b, tag)] = (eg, e01)

            def dpath2(b, tag, obase):
                ft = ft_t[b]
                ftp = ft[:, 0, 0].ap[0]
                eg, e01 = eg_t[(b, tag)]
                egp = eg[:, 0, 0, 0].ap[0]
                nc.vector.tensor_tensor(
                    AP(ft.tensor, ft[:, 0, 0].offset + obase * ext,
                       [ftp, [ext, 2], [1, ext]]),
                    e01[:, :, :],
                    AP(eg.tensor, eg[:, 0, 2, 0].offset,
                       [egp, [cg, 2], [1, ext]]),
                    _alu("mult"),
                )
                del eg_t[(b, tag)]

            def derive(j, ii):
                # derive group o' = 4+ii of block j via permute matmuls
                ft = ft_t[j]
                pm_order = [[0, 1, 2], [2, 1, 0], [0, 1, 2]]
                op_ = 4 + ii
                os_, sh_ = 6 - op_, op_ - 3
                pd = ppsum.tile([P, xw], dt.float32, tag="pd", name="pd")
                pres = [i for i in pm_order[ii] if 0 <= j + (i - 1) < nb]
                for k_i, i in enumerate(pres):
                    src_ft = ft_t[j + (i - 1)]
                    st_, sp_ = (k_i == 0), (k_i == len(pres) - 1)
                    for seg0, seg1 in ((0, MMN), (MMN, xw)):
                        nc.tensor.matmul(
                            pd[:, seg0:seg1],
                            pm[:, i, :],
                            src_ft[:, os_, sh_ + seg0: sh_ + seg1],
                            start=st_, stop=sp_,
                        )
                nc.scalar.copy(ft[:, op_, 0:xw], pd[:, :])

            def process(j):
                ft = ft_t[j]
                ftp = ft[:, 0, 0].ap[0]
                stx = stx_t[j]
                # pp first so it lands at psum offset 0 (bank-aligned segs)
                pp = ppsum.tile([RB, 3072], dt.float32, tag="pp", name="pp")
                pp_t[j] = pp
                edge = (j == 0) or (j == nb - 1)
                # psum layout: V [0,1920) banks 0-3, pad [1920,2048),
                # den [2048,2688) banks 4-5. src_a = col in v/fv coords.
                segs = [(0, 512, 0), (512, 1024, 512), (1024, 1536, 1024),
                        (1536, 1920, 1536), (2048, 2560, 1920),
                        (2560, 2688, 2432)]
                # V-bundles for source groups 0..3 (no cross-block deps):
                # these fill the DVE while block j+1's EG/e01 chain runs.
                for o in range(4):
                    vbundle(j, o, pp, segs, edge)

            def vbundle(j, o, pp, segs, edge):
                ft = ft_t[j]
                ftp = ft[:, 0, 0].ap[0]
                stx = stx_t[j]
                v = pvv.tile([P, C, xw], dt.float16, tag="v", name="v")
                f_ap = AP(ft.tensor, ft[:, 0, 0].offset + o * ext,
                          [ftp, [0, C], [1, xw]])
                s_ap = AP(stx.tensor, stx[:, 0, 3].offset + o,
                          [stx[:, 0, 0].ap[0], [sw, C], [1, xw]])
                nc.vector.tensor_tensor(v[:, :, :], f_ap, s_ap, _alu("mult"))
                for a, b_, sa in segs:
                    if a < vgw:
                        rhs = AP(v.tensor, v[:, 0, 0].offset + sa,
                                 [v[:, 0, 0].ap[0], [1, b_ - a]])
                    else:
                        rhs = ft[:, o, sa - vgw: sa - vgw + b_ - a]
                    nc.tensor.matmul(
                        pp[:, a:b_], col[:, o, :], rhs,
                        start=(o == 0),
                        stop=(o == K - 1 and not edge),
                    )

            def process2a(j):
                pp = pp_t[j]
                edge = (j == 0) or (j == nb - 1)
                segs = [(0, 512, 0), (512, 1024, 512), (1024, 1536, 1024),
                        (1536, 1920, 1536), (2048, 2560, 1920),
                        (2560, 2688, 2432)]
                derive(j, 0)
                vbundle(j, 4, pp, segs, edge)

            def process2b(j):
                pp = pp_t[j]
                edge = (j == 0) or (j == nb - 1)
                segs = [(0, 512, 0), (512, 1024, 512), (1024, 1536, 1024),
                        (1536, 1920, 1536), (2048, 2560, 1920),
                        (2560, 2688, 2432)]
                for o in (5, 6):
                    derive(j, o - 4)
                    vbundle(j, o, pp, segs, edge)
                if edge:
                    fc_, fv_ = (fcol, fv) if j == 0 else (fcolb, fvb)
                    for a, b_, sa in segs:
                        nc.tensor.matmul(
                            pp[:, a:b_], fc_[0:RB, :],
                            fv_[0:RB, sa: sa + b_ - a],
                            start=False, stop=True,
                        )

                # ---- evacuate + gather (stage packs [V | den] tightly)
                g, idx = j // 7, j % 7
                stg = pstage.tile([RB, fw], dt.float16, tag="stg", name="stg")
                nc.scalar.copy(stg[:, 0:vgw], pp[:, 0:vgw])
                nc.scalar.copy(stg[:, vgw:fw], pp[:, 2048:2688])
                rows = slice(idx * RB, (idx + 1) * RB)
                nc.sync.dma_start(gat[g][rows, :], stg[:, :])
                del pp_t[j], stx_t[j], cc_t[j]
                if j == 7 * g + 6 or j == nb - 1:
                    finalize(g)

            def finalize(g):
                rg = grp_rows[g]
                den32 = pfin.tile([P, xw], dt.float32, tag="den32", name="den32")
                nc.vector.tensor_copy(den32[0:rg, :], gat[g][0:rg, vgw:fw])
                rec32 = pfin.tile([P, xw], dt.float32, tag="rec32", name="rec32")
                nc.vector.reciprocal_approx_fast(rec32[0:rg, :], den32[0:rg, :])
                rec16 = pfin.tile([P, xw], dt.float16, tag="rec16", name="rec16")
                nc.vector.tensor_copy(rec16[0:rg, :], rec32[0:rg, :])
                for c in range(C):
                    ot = pfin.tile([P, xw], dt.float32, tag="ot", name="ot")
                    nc.gpsimd.tensor_tensor(
                        ot[0:rg, :], gat[g][0:rg, c * xw:(c + 1) * xw],
                        rec16[0:rg, :], _alu("mult"),
                    )
                    nc.sync.dma_start(OUT[c, g * P: g * P + rg, :], ot[0:rg, :])

            for b in range(nb):
                load_dpath(b)
                if b >= 1:
                    process(b - 1)
                dpath2(b, "A", 2)
                if b >= 1:
                    process2a(b - 1)
                dpath2(b, "B", 0)
                if b >= 1:
                    process2b(b - 1)
            process(nb - 1)
            process2a(nb - 1)
            process2b(nb - 1)

    nc.compile()
    return nc


def _gw7():
    ax = np.arange(-K // 2 + 1.0, K // 2 + 1.0)
    xx, yy = np.meshgrid(ax, ax)
    kern = np.exp(-(xx**2 + yy**2) / (2.0 * 5.0**2))
    return (kern / (1.0 / (2 * math.pi * 5.0**2))).astype(np.float64)


def host_prepare(I: np.ndarray, gw49: np.ndarray):
    """I: (1, C, Him, Wim) fp32 -> per-core input dicts."""
    _, c_, him, wim = I.shape
    nb = him // (HSH * RB)
    xw = wim // WSH
    sw = xw + 12
    rs = nb * RB

    gw7 = gw49.reshape(K, K).astype(np.float64)

    Ip = np.zeros((C, him + 2 * PAD, wim + 12), dtype=F32)
    Ip[:, PAD: PAD + him, 6: 6 + wim] = I[0]
    Ib = Ip.astype(F16)

    # collapse weights: COL[(jy,r), o, r] = gw7[jy, o]
    colw = np.zeros((P, K, RB), dtype=F16)
    for jy in range(JY):
        for r in range(RB):
            colw[jy * RB + r, :, r] = gw7[jy, :].astype(F16)

    # permute matrices PMm/PM0/PMp: PM[p_src, i, p']
    pmw = np.zeros((P, 3, P), dtype=F16)
    for jyp in range(JY):
        for r in range(RB):
            pp_ = jyp * RB + r
            rs_ = r + jyp - 3
            base = (6 - jyp) * RB
            if rs_ < 0:
                pmw[base + rs_ + RB, 0, pp_] = 1.0
            elif rs_ < RB:
                pmw[base + rs_, 1, pp_] = 1.0
            else:
                pmw[base + rs_ - RB, 2, pp_] = 1.0

    in_maps = []
    for i in range(N_CORES):
        hi, wi = i // WSH, i % WSH
        sh = Ib[:, rs * hi: rs * hi + rs + 2 * PAD, xw * wi: xw * wi + sw]
        s0, s1, s2 = sh.strides
        w1 = np.lib.stride_tricks.as_strided(
            sh, shape=(C, nb, JY, RB, sw),
            strides=(s0, RB * s1, s1, s1, s2),
        )
        STa = np.ascontiguousarray(
            w1.transpose(1, 2, 3, 0, 4)).reshape(nb, P, C, sw)

        shp = np.zeros((C, rs + 2 * PAD, sw + 3), dtype=F16)
        shp[:, :, :sw] = sh
        frs = np.zeros((36, C, EXT), dtype=F16)
        frc = np.zeros((36, C, EXT), dtype=F16)
        fcol = np.zeros((36, RB), dtype=F16)
        for t, (dx, (r, dy)) in enumerate(
            [(dx, rd) for dx in (1, 2, 3) for rd in _TOP_RD]
        ):
            frs[t] = shp[:, PAD + r + dy, 6 + dx: 6 + dx + EXT]
            frc[t] = shp[:, PAD + r, 6: 6 + EXT]
            fcol[t, r] = gw7[dy + 3, dx + 3]
        for t, (dx, (r, dy)) in enumerate(
            [(dx, rd) for dx in (1, 2, 3) for rd in _BOT_RD]
        ):
            rr = (nb - 1) * RB + r
            frs[18 + t] = shp[:, PAD + rr + dy, 6 + dx: 6 + dx + EXT]
            frc[18 + t] = shp[:, PAD + rr, 6: 6 + EXT]
            fcol[18 + t, r] = gw7[dy + 3, dx + 3]

        in_maps.append({
            "STX": STa, "COL": colw, "PM": pmw,
            "FRS": frs, "FRC": frc, "FCOL": fcol,
        })
    return in_maps, nb, xw, rs


def assemble(results, him, wim, rs, xw):
    out = np.empty((1, C, him, wim), dtype=F32)
    for i in range(N_CORES):
        hi, wi = i // WSH, i % WSH
        out[0, :, rs * hi: rs * hi + rs, xw * wi: xw * wi + xw] = \
            results[i]["OUT"]
    return out


def _numpy_fallback(I, g):
    n, c, h, w = I.shape
    Ipad = np.zeros((n, c, h + 2 * PAD, w + 2 * PAD), dtype=np.float64)
    Ipad[:, :, PAD: PAD + h, PAD: PAD + w] = I
    num = np.zeros((n, c, h, w), dtype=np.float64)
    den = np.zeros((n, h, w), dtype=np.float64)
    g64 = g.astype(np.float64)
    for j in range(K * K):
        dy, dx = j // K, j % K
        S = Ipad[:, :, dy: dy + h, dx: dx + w]
        D = ((S - I.astype(np.float64)) ** 2).sum(axis=1)
        wgt = np.exp(EXP_SCALE * D) * NORM_COLOR * g64[:, j]
        num += wgt[:, None] * S
        den += wgt
    return (num / den[:, None]).astype(F32)


_CACHE = {}
TRACE = False
LAST_EXEC_NS = None
_LDW_PATCHED = False


def _enable_ldw_prune():
    global _LDW_PATCHED
    if _LDW_PATCHED:
        return
    import json as _json
    import concourse.bass_utils as _bu

    _orig = _bu.compile_bir_kernel

    def _prune(bir_json):
        js = _json.loads(bir_json)
        for fn in js.get("functions", []):
            for blk in fn.get("blocks", []):
                insts = blk.get("instructions", [])
                out = []
                last_ldw = None
                for inst in insts:
                    if inst.get("opcode") == "Ldweights":
                        si = inst.get("sync_info") or {}
                        key = _json.dumps(inst.get("ins"), sort_keys=True)
                        if (last_ldw == key and not si.get("on_wait")
                                and not si.get("on_update")):
                            continue
                        last_ldw = key
                    out.append(inst)
                blk["instructions"] = out
        return _json.dumps(js).encode()

    def _patched(bir_json, tmpdir, neff_name="file.neff"):
        try:
            bir_json = _prune(bir_json)
        except Exception:
            pass
        return _orig(bir_json, tmpdir, neff_name=neff_name)

    _bu.compile_bir_kernel = _patched
    try:
        import concourse.bass2jax as _b2j
        if getattr(_b2j, "compile_bir_kernel", None) is not None:
            _b2j.compile_bir_kernel = _patched
    except Exception:
        pass
    _LDW_PATCHED = True


def kernel(I: np.ndarray, g: np.ndarray) -> np.ndarray:
    global LAST_EXEC_NS
    I = np.asarray(I, dtype=F32)
    g = np.asarray(g)

    gw49 = np.asarray(g[0, :, 0, 0], dtype=F32)
    if not np.array_equal(
        np.asarray(g), np.broadcast_to(np.asarray(g)[:, :, :1, :1], g.shape)
    ):
        return _numpy_fallback(I, g)

    from concourse.bass_utils import run_bass_kernel_spmd
    import os as _os
    if _os.environ.get("BASS_LDW_PRUNE", "1") == "1":
        _enable_ldw_prune()

    in_maps, nb, xw, rs = host_prepare(I, gw49)
    key = (nb, xw)
    if key not in _CACHE:
        _CACHE[key] = build_nc(nb, xw)
    nc = _CACHE[key]
    res = run_bass_kernel_spmd(
        nc, in_maps, core_ids=list(range(N_CORES)), trace=TRACE
    )
    LAST_EXEC_NS = res.exec_time_ns
    return assemble(res.results, I.shape[2], I.shape[3], rs, xw)


def _numpy_mirror_square(I, gw49):
    """Mirror of the device algorithm with Square standing in for the
    gaussian (for CoreSim validation of the AP/permute/fringe machinery)."""
    n, c, h, w = I.shape
    gw7 = gw49.reshape(K, K).astype(np.float64)
    Ipad = np.zeros((n, c, h + 2 * PAD, w + 2 * PAD), dtype=np.float64)
    Ipad[:, :, PAD: PAD + h, PAD: PAD + w] = I
    num = np.zeros((n, c, h, w), dtype=np.float64)
    den = np.zeros((n, h, w), dtype=np.float64)
    for j in range(K * K):
        dy, dx = j // K, j % K
        S = Ipad[:, :, dy: dy + h, dx: dx + w]
        diff = S - I.astype(np.float64)
        E = np.square(math.sqrt(0.5) * diff)
        wgt = E.prod(axis=1) * gw7[dy, dx]
        num += wgt[:, None] * S
        den += wgt
    return (num / den[:, None]).astype(F32)


if __name__ == "__main__":
    import concourse.bass_interp as bass_interp

    globals()["ACT_FUNC"] = "Square"
    globals()["ACT_SCALE"] = math.sqrt(0.5)

    rng = np.random.default_rng(0)
    him, wim = HSH * RB * 3, W  # 3 blocks per core
    I = rng.random((1, C, him, wim), dtype=F32)
    gw49 = _gw7().reshape(-1).astype(F32)

    in_maps, nb, xw, rs = host_prepare(I, gw49)
    nc = build_nc(nb, xw)
    sim = bass_interp.CoreSim(nc)
    for k, v in in_maps[0].items():
        sim.tensor(k)[:] = v
    sim.simulate()
    got = np.array(sim.tensor("OUT"))

    exp_full = _numpy_mirror_square(I, gw49)
    exp0 = exp_full[0, :, 0:rs, 0:xw]
    err = np.abs(got - exp0)
    print("sim err max:", err.max(), "rel:", err.max() / np.abs(exp0).max())
    # per-region check to localize issues
    for name, sl in [("top3rows", np.s_[:, 0:3, :]),
                     ("bot3rows", np.s_[:, rs - 3: rs, :]),
                     ("mid", np.s_[:, 3: rs - 3, :]),
                     ("blk-edge", np.s_[:, 16:20, :])]:
        e = np.abs(got[sl] - exp0[sl]).max()
        print(f"  {name:10s} max err {e:.5f}")
